# revision 22
# baseline (speedup 1.0000x reference)
"""Trainium2 Bass kernel for CausalSelfAttentionLast (last-query attention).

Reference math (per batch b):
    k = x @ Wk + bk                      [T, C]
    v = x @ Wv + bv                      [T, C]
    q = x[-1] @ Wq + bq                  [C]
    wei = softmax((q . k[t]) / sqrt(C))  [T]
    out = (wei @ v) @ Wp + bp            [C]

Algebraic restructuring (exact math, avoids the O(T*C^2) K/V projections):
    q . k[t] = x[t] . (Wk @ q) + q . bk
      -> u := Wk @ q  [C];  the q.bk term is constant in t and cancels in
         softmax, so bk never enters the computation at all.
    wei @ v = ((wei @ x) @ Wv) + (sum_t wei) * bv = (wei @ x) @ Wv + bv
    out = ((wei @ x) @ Wv + bv) @ Wp + bp

This reduces ~275 GFLOP of projections to ~0.3 GFLOP: a single streaming
pass over x computing logits[t] = scale * x[t].u and P[c] = sum_t e[t] x[t,c]
(unnormalized), plus tiny [4,C] x [C,C] matvec batches.

Sharding over 8 cores:
  - x is split along T (512 positions per core); each core computes its
    logits chunk and partial P/sum-of-exp.
  - the 4 CxC weights are split along the output (or contraction) dim in
    256-column slices per core; two 32KB AllReduces combine u and [P|Z].
  - final (wx @ Wv) @ Wp is sharded over the 256-wide inner dim; host sums
    the 8 partial outputs.
"""

import os
from contextlib import ExitStack

import numpy as np

import concourse.bass as bass
import concourse.tile as tile
from concourse import bacc, mybir
from concourse.bass_utils import run_bass_kernel_spmd
from concourse.masks import make_identity

B, T, C = 4, 4096, 2048
NC = 8
TS = T // NC        # 512 sequence positions per core
CS = C // NC        # 256-wide weight slice per core
KK = C // 128       # 16 contraction chunks of 128
P = 128
SCALE = float(C) ** -0.5
F32 = mybir.dt.float32
AF = mybir.ActivationFunctionType
ALU = mybir.AluOpType

# tuning knobs (env-overridable for cost-model sweeps; defaults are shipped)
XPOOL_BUFS = int(os.environ.get("K_XPOOL_BUFS", "7"))
SCRATCH_BUFS = int(os.environ.get("K_SCRATCH_BUFS", "1"))
ABL = set(os.environ.get("K_ABL", "").split(",")) - {""}
# float32r (single-pass fp32 matmul, 4x faster, reduced precision) for the
# P-accumulation matmuls; default off pending accuracy verification
USE_F32R = os.environ.get("K_F32R", "0") == "1"
F32R = mybir.dt.float32r


def _emit(nc, tc, ctx, io, with_cc, nd):
    persist = ctx.enter_context(tc.tile_pool(name="persist", bufs=1))
    psum = ctx.enter_context(tc.tile_pool(name="psum", bufs=8, space="PSUM"))
    dram = ctx.enter_context(tc.tile_pool(name="dram", bufs=1, space="DRAM"))
    xpool = ctx.enter_context(tc.tile_pool(name="xp", bufs=XPOOL_BUFS))
    scratch = ctx.enter_context(tc.tile_pool(name="scr", bufs=SCRATCH_BUFS))
    psmall = ctx.enter_context(tc.tile_pool(name="psb", bufs=2))

    def cc_allreduce(cin, cout):
        if with_cc:
            nc.gpsimd.collective_compute(
                "AllReduce",
                ALU.add,
                replica_groups=[list(range(nd))],
                ins=[cin.opt()],
                outs=[cout.opt()],
            )
        else:
            nc.gpsimd.dma_start(cout[:], cin[:])

    # --- small constants (SWDGE ring, out of the way of bulk loads) ---
    xlast_sb = persist.tile([P, KK * B], F32, tag="xlast")  # [c_chunk -> (k b)]
    nc.gpsimd.dma_start(
        xlast_sb[:].rearrange("p (k b) -> p k b", k=KK),
        io["xlastT"].rearrange("(k p) b -> p k b", p=P),
    )
    bq_sb = persist.tile([P, 2], F32, tag="bq")
    nc.gpsimd.dma_start(
        bq_sb[:].rearrange("p (m o) -> p m o", m=2),
        io["bqT"].rearrange("(m p) o -> p m o", p=P),
    )
    bv_sb = persist.tile([P, 2], F32, tag="bv")
    nc.gpsimd.dma_start(
        bv_sb[:].rearrange("p (m o) -> p m o", m=2),
        io["bvT"].rearrange("(m p) o -> p m o", p=P),
    )
    sel_sb = persist.tile([16, B], F32, tag="sel")
    nc.gpsimd.dma_start(sel_sb[:], io["sel"][:, :])
    selT_sb = persist.tile([B, 16], F32, tag="selT")
    nc.gpsimd.dma_start(selT_sb[:], io["selT"][:, :])
    bp_sb = persist.tile([B, C], F32, tag="bp")
    nc.gpsimd.dma_start(bp_sb[:], io["bp_bc"][:, :])
    ones_sb = persist.tile([P, 1], F32, tag="ones")
    nc.gpsimd.memset(ones_sb[:], 1.0)
    ident_sb = persist.tile([B, B], F32, tag="ident")
    make_identity(nc, ident_sb[:])
    e_sb = persist.tile([P, 16], F32, tag="e")  # exp(logits), col = b*4+t4

    # --- phase A: q = Wq^T x_last (transposed), u = Wk @ q, AllReduce u ---
    qT_list = []
    u_sb = persist.tile([B, C], F32, tag="u")
    with tc.tile_pool(name="wA", bufs=1) as wA:
        wq_sb = wA.tile([P, KK * CS], F32, tag="wq")
        for k in range(KK):
            nc.sync.dma_start(wq_sb[:, k * CS:(k + 1) * CS], io["wq"][k * P:(k + 1) * P, :])
        wkT_sb = wA.tile([P, 2 * C], F32, tag="wkT")
        for k in range(2):
            nc.sync.dma_start(wkT_sb[:, k * C:(k + 1) * C], io["wkT"][k * P:(k + 1) * P, :])

        if os.environ.get("K_QSTYLE", "narrow") == "wide":
            # q[b, c'] = sum_c x_last[b, c] Wq[c, c'] as one [4, 256] accum
            q_ps = psum.tile([B, CS], F32, tag="ps")
            for k in range(KK):
                nc.tensor.matmul(
                    q_ps[:],
                    xlast_sb[:, k * B:(k + 1) * B],
                    wq_sb[:, k * CS:(k + 1) * CS],
                    start=(k == 0),
                    stop=(k == KK - 1),
                )
            q_sb = persist.tile([B, CS], F32, tag="q")
            nc.scalar.copy(q_sb[:], q_ps[:])
            for m in range(2):
                qt_ps = psum.tile([P, B], F32, tag="ps", name=f"qt_ps_{m}")
                nc.tensor.transpose(qt_ps[:], q_sb[:, m * P:(m + 1) * P], ident_sb[:])
                qT_sb = persist.tile([P, B], F32, tag=f"qT{m}", name=f"qT_sb_{m}")
                # bias bq folds in after the transpose (per-partition there)
                nc.scalar.activation(
                    qT_sb[:], qt_ps[:], AF.Identity, bias=bq_sb[:, m:m + 1], scale=1.0
                )
                qT_list.append(qT_sb)
        else:
            for m in range(2):
                q_ps = psum.tile([P, B], F32, tag="ps", name=f"q_ps_{m}")
                for k in range(KK):
                    nc.tensor.matmul(
                        q_ps[:],
                        wq_sb[:, k * CS + m * P: k * CS + m * P + P],
                        xlast_sb[:, k * B:(k + 1) * B],
                        start=(k == 0),
                        stop=(k == KK - 1),
                    )
                qT_sb = persist.tile([P, B], F32, tag=f"qT{m}", name=f"qT_sb_{m}")
                nc.scalar.activation(
                    qT_sb[:], q_ps[:], AF.Identity, bias=bq_sb[:, m:m + 1], scale=1.0
                )
                qT_list.append(qT_sb)

        for n in range(4):
            u_ps = psum.tile([B, 512], F32, tag="ps")
            for k in range(2):
                nc.tensor.matmul(
                    u_ps[:],
                    qT_list[k][:],
                    wkT_sb[:, k * C + n * 512: k * C + (n + 1) * 512],
                    start=(k == 0),
                    stop=(k == 1),
                )
            nc.scalar.copy(u_sb[:, n * 512:(n + 1) * 512], u_ps[:])

    cc1_in = dram.tile([B, C], F32, tag="cc1i")
    cc1_out = dram.tile([B, C], F32, tag="cc1o")
    nc.gpsimd.dma_start(cc1_in[:], u_sb[:])
    cc_allreduce(cc1_in, cc1_out)

    # --- phase B: stream x; logits -> exp -> P accumulation ---
    cc2_in = dram.tile([1, B * C + B], F32, tag="cc2i")
    cc2_out = dram.tile([1, B * C + B], F32, tag="cc2o")
    poolB = None
    wv_sb = wp_sb = None
    for b in range(B):
        # bring back this b's row of the AllReduced u at partition 0, then
        # broadcast it across all 128 partitions for the DVE dot products
        urow_sb = psmall.tile([1, C], F32, tag="urow", name=f"urow_{b}")
        nc.gpsimd.dma_start(urow_sb[:], cc1_out[b:b + 1, :])
        u_bc = psmall.tile([P, C], F32, tag="ubc", name=f"ubc_{b}")
        nc.gpsimd.partition_broadcast(u_bc[:], urow_sb[:1, :])
        lg_sb = psmall.tile([P, 4], F32, tag="lg", name=f"lg_{b}")
        p_ps = [
            psum.tile([1, 512], F32, tag="ps", name=f"p_ps_{b}_{n}") for n in range(4)
        ]
        for t4 in range(4):
            tt = b * 4 + t4
            x_sb = xpool.tile([P, C], F32R if USE_F32R else F32, tag="x")
            nc.sync.dma_start(
                x_sb[:],
                io["xs"][tt * P:(tt + 1) * P, :].bitcast(F32R)
                if USE_F32R
                else io["xs"][tt * P:(tt + 1) * P, :],
            )
            x_f32 = x_sb[:].bitcast(F32) if USE_F32R else x_sb[:]
            if "nostt" not in ABL:
                tmp = scratch.tile([P, C], F32, tag="tmp")
                # fused (x * scale) * u with free-dim sum -> scaled logits
                # column (tensor_tensor_reduce is unsupported on this runtime)
                nc.vector.scalar_tensor_tensor(
                    out=tmp[:],
                    in0=x_f32,
                    scalar=SCALE,
                    in1=u_bc[:],
                    op0=ALU.mult,
                    op1=ALU.mult,
                    accum_out=lg_sb[:, t4:t4 + 1],
                )
            nc.scalar.activation(e_sb[:, tt:tt + 1], lg_sb[:, t4:t4 + 1], AF.Exp)
            if USE_F32R:
                er_col = psmall.tile([P, 1], F32R, tag="er", name=f"er_{tt}")
                nc.vector.tensor_copy(er_col[:], e_sb[:, tt:tt + 1])
                lhs_col = er_col[:]
            else:
                lhs_col = e_sb[:, tt:tt + 1]
            if "nopmm" not in ABL:
                for n in range(4):
                    nc.tensor.matmul(
                        p_ps[n][:],
                        lhs_col,
                        x_sb[:, n * 512:(n + 1) * 512],
                        start=(t4 == 0),
                        stop=(t4 == 3),
                    )
        pb_sb = psmall.tile([1, C], F32, tag="pb", bufs=1, name=f"pb_{b}")
        for n in range(4):
            nc.scalar.copy(pb_sb[:, n * 512:(n + 1) * 512], p_ps[n][:1, :])
        nc.scalar.dma_start(cc2_in[:, b * C:(b + 1) * C], pb_sb[:])

        if b == 0:
            # late bulk weight loads overlap the x stream on the same ring
            poolB = ctx.enter_context(tc.tile_pool(name="wB", bufs=1))
            wv_sb = poolB.tile([P, KK * CS], F32, tag="wv")
            for k in range(KK):
                nc.sync.dma_start(
                    wv_sb[:, k * CS:(k + 1) * CS], io["wv"][k * P:(k + 1) * P, :]
                )
            wp_sb = poolB.tile([P, 2 * C], F32, tag="wp")
            for k in range(2):
                nc.sync.dma_start(
                    wp_sb[:, k * C:(k + 1) * C], io["wp"][k * P:(k + 1) * P, :]
                )

    # --- sum of exp per b: column sums then fold groups of 4 ---
    cs_ps = psum.tile([16, 1], F32, tag="ps")
    nc.tensor.matmul(cs_ps[:], e_sb[:], ones_sb[:], start=True, stop=True)
    cs_sb = psmall.tile([16, 1], F32, tag="cs")
    nc.scalar.copy(cs_sb[:], cs_ps[:])
    srow_ps = psum.tile([1, B], F32, tag="ps")
    nc.tensor.matmul(srow_ps[:], cs_sb[:], sel_sb[:], start=True, stop=True)
    srow_sb = psmall.tile([1, B], F32, tag="srow")
    nc.scalar.copy(srow_sb[:], srow_ps[:])
    nc.scalar.dma_start(cc2_in[:, B * C:], srow_sb[:])

    cc_allreduce(cc2_in, cc2_out)
    wx4_sb = persist.tile([B, C], F32, tag="wx4")
    nc.scalar.dma_start(
        wx4_sb[:], cc2_out[:, :B * C].rearrange("o (b c) -> (o b) c", b=B)
    )
    z4_sb = persist.tile([B, 1], F32, tag="z4")
    nc.scalar.dma_start(
        z4_sb[:], cc2_out[:, B * C:].rearrange("o (b c) -> (o b) c", b=B)
    )

    # --- wei output: e / Z ---
    recip_sb = persist.tile([B, 1], F32, tag="rz")
    nc.vector.reciprocal(recip_sb[:], z4_sb[:])
    rz_ps = psum.tile([1, 16], F32, tag="ps")
    nc.tensor.matmul(rz_ps[:], recip_sb[:], selT_sb[:], start=True, stop=True)
    rzrow_sb = psmall.tile([1, 16], F32, tag="rzrow")
    nc.scalar.copy(rzrow_sb[:], rz_ps[:])
    rzbc_sb = persist.tile([P, 16], F32, tag="rzbc")
    nc.gpsimd.partition_broadcast(rzbc_sb[:], rzrow_sb[:1, :])
    wei_sb = persist.tile([P, 16], F32, tag="wei")
    nc.vector.tensor_mul(wei_sb[:], e_sb[:], rzbc_sb[:])
    nc.scalar.dma_start(io["wei_o"][:, :], wei_sb[:])

    # --- final: y_partial = ((P/Z) @ Wv_slice + bv_slice) @ Wp_slice (+bp) ---
    wx_sb = wx4_sb  # normalize in place: wx = P / Z
    nc.scalar.activation(wx_sb[:], wx4_sb[:], AF.Copy, scale=recip_sb[:, 0:1])
    wxT_sb = persist.tile([P, KK * B], F32, tag="wxT")
    for j in range(KK):
        tp_ps = psum.tile([P, B], F32, tag="ps")
        nc.tensor.transpose(tp_ps[:], wx_sb[:, j * P:(j + 1) * P], ident_sb[:])
        nc.vector.tensor_copy(wxT_sb[:, j * B:(j + 1) * B], tp_ps[:])

    tT_list = []
    if os.environ.get("K_TSTYLE", "narrow") == "wide":
        # t[b, c1] = sum_c wx[b, c] Wv[c, c1] as one [4, 256] accumulation
        t_ps = psum.tile([B, CS], F32, tag="ps")
        for k in range(KK):
            nc.tensor.matmul(
                t_ps[:],
                wxT_sb[:, k * B:(k + 1) * B],
                wv_sb[:, k * CS:(k + 1) * CS],
                start=(k == 0),
                stop=(k == KK - 1),
            )
        t_sb = persist.tile([B, CS], F32, tag="t")
        nc.scalar.copy(t_sb[:], t_ps[:])
        for m in range(2):
            tt_ps = psum.tile([P, B], F32, tag="ps", name=f"tt_ps_{m}")
            nc.tensor.transpose(tt_ps[:], t_sb[:, m * P:(m + 1) * P], ident_sb[:])
            tT_sb = persist.tile([P, B], F32, tag=f"tT{m}", name=f"tT_sb_{m}")
            nc.scalar.activation(
                tT_sb[:], tt_ps[:], AF.Identity, bias=bv_sb[:, m:m + 1], scale=1.0
            )
            tT_list.append(tT_sb)
    else:
        for m in range(2):
            t2_ps = psum.tile([P, B], F32, tag="ps", name=f"t2_ps_{m}")
            for k in range(KK):
                nc.tensor.matmul(
                    t2_ps[:],
                    wv_sb[:, k * CS + m * P: k * CS + m * P + P],
                    wxT_sb[:, k * B:(k + 1) * B],
                    start=(k == 0),
                    stop=(k == KK - 1),
                )
            tT_sb = persist.tile([P, B], F32, tag=f"tT{m}", name=f"tT_sb_{m}")
            nc.scalar.activation(
                tT_sb[:], t2_ps[:], AF.Identity, bias=bv_sb[:, m:m + 1], scale=1.0
            )
            tT_list.append(tT_sb)

    for n in range(4):
        y_ps = psum.tile([B, 512], F32, tag="ps", name=f"y_ps_{n}")
        for k in range(2):
            nc.tensor.matmul(
                y_ps[:],
                tT_list[k][:],
                wp_sb[:, k * C + n * 512: k * C + (n + 1) * 512],
                start=(k == 0),
                stop=(k == 1),
            )
        y_sb = psmall.tile([B, 512], F32, tag="yp", name=f"y_sb_{n}")
        nc.vector.tensor_add(y_sb[:], y_ps[:], bp_sb[:, n * 512:(n + 1) * 512])
        nc.scalar.dma_start(io["y_o"][:, n * 512:(n + 1) * 512], y_sb[:])


def build(n_devices=NC, with_cc=True):
    nc = bacc.Bacc(
        "TRN2",
        target_bir_lowering=False,
        debug=False,
        enable_asserts=True,
        num_devices=n_devices,
    )
    io = {}
    for name, shape in [
        ("xs", [B * TS, C]),
        ("xlastT", [C, B]),
        ("wq", [C, CS]),
        ("bqT", [CS, 1]),
        ("wkT", [CS, C]),
        ("wv", [C, CS]),
        ("bvT", [CS, 1]),
        ("wp", [CS, C]),
        ("bp_bc", [B, C]),
        ("sel", [16, B]),
        ("selT", [B, 16]),
    ]:
        io[name] = nc.dram_tensor(name, shape, F32, kind="ExternalInput").ap()
    for name, shape in [("y_o", [B, C]), ("wei_o", [P, 16])]:
        io[name] = nc.dram_tensor(name, shape, F32, kind="ExternalOutput").ap()

    with tile.TileContext(nc) as tc:
        with ExitStack() as ctx:
            _emit(nc, tc, ctx, io, with_cc, n_devices)
    nc.compile()
    return nc


_NC_CACHE = {}


def _get_nc():
    if "nc" not in _NC_CACHE:
        _NC_CACHE["nc"] = build()
    return _NC_CACHE["nc"]


def make_in_maps(x, Wk, bk, Wq, bq, Wv, bv, Wp, bp):
    x = np.ascontiguousarray(np.asarray(x, np.float32))
    xlastT = np.ascontiguousarray(x[:, -1, :].T)  # [C, B]
    sel = np.zeros((16, B), np.float32)
    for j in range(16):
        sel[j, j // 4] = 1.0
    selT = np.ascontiguousarray(sel.T)
    in_maps = []
    for i in range(NC):
        sl = slice(i * CS, (i + 1) * CS)
        in_maps.append({
            "xs": np.ascontiguousarray(
                x[:, i * TS:(i + 1) * TS, :].reshape(B * TS, C)
            ),
            "xlastT": xlastT,
            "wq": np.ascontiguousarray(np.asarray(Wq, np.float32)[:, sl]),
            "bqT": np.ascontiguousarray(np.asarray(bq, np.float32)[sl, None]),
            "wkT": np.ascontiguousarray(np.asarray(Wk, np.float32)[:, sl].T),
            "wv": np.ascontiguousarray(np.asarray(Wv, np.float32)[:, sl]),
            "bvT": np.ascontiguousarray(np.asarray(bv, np.float32)[sl, None]),
            "wp": np.ascontiguousarray(np.asarray(Wp, np.float32)[sl, :]),
            "bp_bc": (
                np.tile(np.asarray(bp, np.float32), (B, 1))
                if i == 0
                else np.zeros((B, C), np.float32)
            ),
            "sel": sel,
            "selT": selT,
        })
    return in_maps


def assemble_outputs(results):
    y = np.zeros((B, C), np.float64)
    wei = np.empty((B, T), np.float32)
    for i in range(NC):
        y += results[i]["y_o"].astype(np.float64)
        w = results[i]["wei_o"]  # [128, 16], col = b*4 + t4
        wei[:, i * TS:(i + 1) * TS] = (
            w.reshape(P, B, 4).transpose(1, 2, 0).reshape(B, TS)
        )
    out_last = y.astype(np.float32).reshape(B, 1, C)
    return out_last, wei.reshape(B, 1, T)


def kernel(**inputs):
    nc = _get_nc()
    in_maps = make_in_maps(**inputs)
    res = run_bass_kernel_spmd(nc, in_maps, core_ids=list(range(NC)))
    return assemble_outputs(res.results)


# revision 23
# speedup vs baseline: 1.1364x; 1.1364x over previous
"""Trainium2 Bass kernel for CausalSelfAttentionLast (last-query attention).

Reference math (per batch b):
    k = x @ Wk + bk                      [T, C]
    v = x @ Wv + bv                      [T, C]
    q = x[-1] @ Wq + bq                  [C]
    wei = softmax((q . k[t]) / sqrt(C))  [T]
    out = (wei @ v) @ Wp + bp            [C]

Algebraic restructuring (exact math, avoids the O(T*C^2) K/V projections):
    q . k[t] = x[t] . (Wk @ q) + q . bk
      -> u := Wk @ q  [C];  the q.bk term is constant in t and cancels in
         softmax, so bk never enters the computation at all.
    wei @ v = ((wei @ x) @ Wv) + (sum_t wei) * bv = (wei @ x) @ Wv + bv
    out = ((wei @ x) @ Wv + bv) @ Wp + bp

This reduces ~275 GFLOP of projections to ~0.3 GFLOP: a single streaming
pass over x computing logits[t] = scale * x[t].u and P[c] = sum_t e[t] x[t,c]
(unnormalized), plus tiny [4,C] x [C,C] matvec batches.

Sharding over 8 cores:
  - x is split along T (512 positions per core); each core computes its
    logits chunk and partial P/sum-of-exp.
  - the 4 CxC weights are split along the output (or contraction) dim in
    256-column slices per core; two 32KB AllReduces combine u and [P|Z].
  - final (wx @ Wv) @ Wp is sharded over the 256-wide inner dim; host sums
    the 8 partial outputs.
"""

import os
from contextlib import ExitStack

import numpy as np

import concourse.bass as bass
import concourse.tile as tile
from concourse import bacc, mybir
from concourse.bass_utils import run_bass_kernel_spmd
from concourse.masks import make_identity

B, T, C = 4, 4096, 2048
NC = 8
TS = T // NC        # 512 sequence positions per core
CS = C // NC        # 256-wide weight slice per core
KK = C // 128       # 16 contraction chunks of 128
P = 128
SCALE = float(C) ** -0.5
F32 = mybir.dt.float32
AF = mybir.ActivationFunctionType
ALU = mybir.AluOpType

# tuning knobs (env-overridable for cost-model sweeps; defaults are shipped)
XPOOL_BUFS = int(os.environ.get("K_XPOOL_BUFS", "7"))
SCRATCH_BUFS = int(os.environ.get("K_SCRATCH_BUFS", "1"))
ABL = set(os.environ.get("K_ABL", "").split(",")) - {""}
# float32r (single-pass fp32 matmul, 4x faster, reduced precision) for the
# P-accumulation matmuls; default off pending accuracy verification
USE_F32R = os.environ.get("K_F32R", "0") == "1"
F32R = mybir.dt.float32r


def _emit(nc, tc, ctx, io, with_cc, nd):
    persist = ctx.enter_context(tc.tile_pool(name="persist", bufs=1))
    psum = ctx.enter_context(tc.tile_pool(name="psum", bufs=8, space="PSUM"))
    dram = ctx.enter_context(tc.tile_pool(name="dram", bufs=1, space="DRAM"))
    xpool = ctx.enter_context(tc.tile_pool(name="xp", bufs=XPOOL_BUFS))
    scratch = ctx.enter_context(tc.tile_pool(name="scr", bufs=SCRATCH_BUFS))
    psmall = ctx.enter_context(tc.tile_pool(name="psb", bufs=2))

    def cc_allreduce(cin, cout):
        if with_cc:
            nc.gpsimd.collective_compute(
                "AllReduce",
                ALU.add,
                replica_groups=[list(range(nd))],
                ins=[cin.opt()],
                outs=[cout.opt()],
            )
        else:
            nc.gpsimd.dma_start(cout[:], cin[:])

    # --- small constants (SWDGE ring, out of the way of bulk loads) ---
    xlast_sb = persist.tile([P, KK * B], F32, tag="xlast")  # [c_chunk -> (k b)]
    nc.gpsimd.dma_start(
        xlast_sb[:].rearrange("p (k b) -> p k b", k=KK),
        io["xlastT"].rearrange("(k p) b -> p k b", p=P),
    )
    bq_sb = persist.tile([P, 2], F32, tag="bq")
    nc.gpsimd.dma_start(
        bq_sb[:].rearrange("p (m o) -> p m o", m=2),
        io["bqT"].rearrange("(m p) o -> p m o", p=P),
    )
    bv_sb = persist.tile([P, 2], F32, tag="bv")
    nc.gpsimd.dma_start(
        bv_sb[:].rearrange("p (m o) -> p m o", m=2),
        io["bvT"].rearrange("(m p) o -> p m o", p=P),
    )
    sel_sb = persist.tile([16, B], F32, tag="sel")
    nc.gpsimd.dma_start(sel_sb[:], io["sel"][:, :])
    selT_sb = persist.tile([B, 16], F32, tag="selT")
    nc.gpsimd.dma_start(selT_sb[:], io["selT"][:, :])
    bp_sb = persist.tile([B, C], F32, tag="bp")
    nc.gpsimd.dma_start(bp_sb[:], io["bp_bc"][:, :])
    ones_sb = persist.tile([P, 1], F32, tag="ones")
    nc.gpsimd.memset(ones_sb[:], 1.0)
    ident_sb = persist.tile([B, B], F32, tag="ident")
    make_identity(nc, ident_sb[:])
    e_sb = persist.tile([P, 16], F32, tag="e")  # exp(logits), col = b*4+t4

    # --- phase A: q = Wq^T x_last (transposed), u = Wk @ q, AllReduce u ---
    qT_list = []
    u_sb = persist.tile([B, C], F32, tag="u")
    with tc.tile_pool(name="wA", bufs=1) as wA:
        wq_sb = wA.tile([P, KK * CS], F32, tag="wq")
        for k in range(KK):
            nc.sync.dma_start(wq_sb[:, k * CS:(k + 1) * CS], io["wq"][k * P:(k + 1) * P, :])
        wkT_sb = wA.tile([P, 2 * C], F32, tag="wkT")
        for k in range(2):
            nc.sync.dma_start(wkT_sb[:, k * C:(k + 1) * C], io["wkT"][k * P:(k + 1) * P, :])

        if os.environ.get("K_QSTYLE", "narrow") == "wide":
            # q[b, c'] = sum_c x_last[b, c] Wq[c, c'] as one [4, 256] accum
            q_ps = psum.tile([B, CS], F32, tag="ps")
            for k in range(KK):
                nc.tensor.matmul(
                    q_ps[:],
                    xlast_sb[:, k * B:(k + 1) * B],
                    wq_sb[:, k * CS:(k + 1) * CS],
                    start=(k == 0),
                    stop=(k == KK - 1),
                )
            q_sb = persist.tile([B, CS], F32, tag="q")
            nc.scalar.copy(q_sb[:], q_ps[:])
            for m in range(2):
                qt_ps = psum.tile([P, B], F32, tag="ps", name=f"qt_ps_{m}")
                nc.tensor.transpose(qt_ps[:], q_sb[:, m * P:(m + 1) * P], ident_sb[:])
                qT_sb = persist.tile([P, B], F32, tag=f"qT{m}", name=f"qT_sb_{m}")
                # bias bq folds in after the transpose (per-partition there)
                nc.scalar.activation(
                    qT_sb[:], qt_ps[:], AF.Identity, bias=bq_sb[:, m:m + 1], scale=1.0
                )
                qT_list.append(qT_sb)
        else:
            for m in range(2):
                q_ps = psum.tile([P, B], F32, tag="ps", name=f"q_ps_{m}")
                for k in range(KK):
                    nc.tensor.matmul(
                        q_ps[:],
                        wq_sb[:, k * CS + m * P: k * CS + m * P + P],
                        xlast_sb[:, k * B:(k + 1) * B],
                        start=(k == 0),
                        stop=(k == KK - 1),
                    )
                qT_sb = persist.tile([P, B], F32, tag=f"qT{m}", name=f"qT_sb_{m}")
                nc.scalar.activation(
                    qT_sb[:], q_ps[:], AF.Identity, bias=bq_sb[:, m:m + 1], scale=1.0
                )
                qT_list.append(qT_sb)

        for n in range(4):
            u_ps = psum.tile([B, 512], F32, tag="ps")
            for k in range(2):
                nc.tensor.matmul(
                    u_ps[:],
                    qT_list[k][:],
                    wkT_sb[:, k * C + n * 512: k * C + (n + 1) * 512],
                    start=(k == 0),
                    stop=(k == 1),
                )
            nc.scalar.copy(u_sb[:, n * 512:(n + 1) * 512], u_ps[:])

    cc1_in = dram.tile([B, C], F32, tag="cc1i")
    cc1_out = dram.tile([B, C], F32, tag="cc1o")
    nc.gpsimd.dma_start(cc1_in[:], u_sb[:])
    cc_allreduce(cc1_in, cc1_out)

    # --- phase B: stream x; logits -> exp -> P accumulation ---
    cc2_in = dram.tile([1, B * C + B], F32, tag="cc2i")
    cc2_out = dram.tile([1, B * C + B], F32, tag="cc2o")
    poolB = None
    wv_sb = wp_sb = None
    for b in range(B):
        # bring back this b's row of the AllReduced u at partition 0, then
        # broadcast it across all 128 partitions for the DVE dot products
        urow_sb = psmall.tile([1, C], F32, tag="urow", name=f"urow_{b}")
        nc.gpsimd.dma_start(urow_sb[:], cc1_out[b:b + 1, :])
        u_bc = psmall.tile([P, C], F32, tag="ubc", name=f"ubc_{b}")
        nc.gpsimd.partition_broadcast(u_bc[:], urow_sb[:1, :])
        lg_sb = psmall.tile([P, 4], F32, tag="lg", name=f"lg_{b}")
        p_ps = [
            psum.tile([1, 512], F32, tag="ps", name=f"p_ps_{b}_{n}") for n in range(4)
        ]
        for t4 in range(4):
            tt = b * 4 + t4
            x_sb = xpool.tile([P, C], F32R if USE_F32R else F32, tag="x")
            nc.sync.dma_start(
                x_sb[:],
                io["xs"][tt * P:(tt + 1) * P, :].bitcast(F32R)
                if USE_F32R
                else io["xs"][tt * P:(tt + 1) * P, :],
            )
            x_f32 = x_sb[:].bitcast(F32) if USE_F32R else x_sb[:]
            if "nostt" not in ABL:
                tmp = scratch.tile([P, C], F32, tag="tmp")
                # fused (x * scale) * u with free-dim sum -> scaled logits
                # column (tensor_tensor_reduce is unsupported on this runtime)
                nc.vector.scalar_tensor_tensor(
                    out=tmp[:],
                    in0=x_f32,
                    scalar=SCALE,
                    in1=u_bc[:],
                    op0=ALU.mult,
                    op1=ALU.mult,
                    accum_out=lg_sb[:, t4:t4 + 1],
                )
            nc.scalar.activation(e_sb[:, tt:tt + 1], lg_sb[:, t4:t4 + 1], AF.Exp)
            if USE_F32R:
                er_col = psmall.tile([P, 1], F32R, tag="er", name=f"er_{tt}")
                nc.vector.tensor_copy(er_col[:], e_sb[:, tt:tt + 1])
                lhs_col = er_col[:]
            else:
                lhs_col = e_sb[:, tt:tt + 1]
            if "nopmm" not in ABL:
                for n in range(4):
                    nc.tensor.matmul(
                        p_ps[n][:],
                        lhs_col,
                        x_sb[:, n * 512:(n + 1) * 512],
                        start=(t4 == 0),
                        stop=(t4 == 3),
                    )
        pb_sb = psmall.tile([1, C], F32, tag="pb", bufs=1, name=f"pb_{b}")
        for n in range(4):
            nc.scalar.copy(pb_sb[:, n * 512:(n + 1) * 512], p_ps[n][:1, :])
        nc.scalar.dma_start(cc2_in[:, b * C:(b + 1) * C], pb_sb[:])

        if b == 0:
            # late bulk weight loads overlap the x stream on the same ring
            poolB = ctx.enter_context(tc.tile_pool(name="wB", bufs=1))
            wv_sb = poolB.tile([P, KK * CS], F32, tag="wv")
            for k in range(KK):
                nc.sync.dma_start(
                    wv_sb[:, k * CS:(k + 1) * CS], io["wv"][k * P:(k + 1) * P, :]
                )
            wp_sb = poolB.tile([P, 2 * C], F32, tag="wp")
            for k in range(2):
                nc.sync.dma_start(
                    wp_sb[:, k * C:(k + 1) * C], io["wp"][k * P:(k + 1) * P, :]
                )

    # --- sum of exp per b: column sums then fold groups of 4 ---
    cs_ps = psum.tile([16, 1], F32, tag="ps")
    nc.tensor.matmul(cs_ps[:], e_sb[:], ones_sb[:], start=True, stop=True)
    cs_sb = psmall.tile([16, 1], F32, tag="cs")
    nc.scalar.copy(cs_sb[:], cs_ps[:])
    srow_ps = psum.tile([1, B], F32, tag="ps")
    nc.tensor.matmul(srow_ps[:], cs_sb[:], sel_sb[:], start=True, stop=True)
    srow_sb = psmall.tile([1, B], F32, tag="srow")
    nc.scalar.copy(srow_sb[:], srow_ps[:])
    nc.scalar.dma_start(cc2_in[:, B * C:], srow_sb[:])

    cc_allreduce(cc2_in, cc2_out)
    wx4_sb = persist.tile([B, C], F32, tag="wx4")
    nc.scalar.dma_start(
        wx4_sb[:], cc2_out[:, :B * C].rearrange("o (b c) -> (o b) c", b=B)
    )
    z4_sb = persist.tile([B, 1], F32, tag="z4")
    nc.scalar.dma_start(
        z4_sb[:], cc2_out[:, B * C:].rearrange("o (b c) -> (o b) c", b=B)
    )

    # --- wei output: e / Z ---
    recip_sb = persist.tile([B, 1], F32, tag="rz")
    nc.vector.reciprocal(recip_sb[:], z4_sb[:])
    rz_ps = psum.tile([1, 16], F32, tag="ps")
    nc.tensor.matmul(rz_ps[:], recip_sb[:], selT_sb[:], start=True, stop=True)
    rzrow_sb = psmall.tile([1, 16], F32, tag="rzrow")
    nc.scalar.copy(rzrow_sb[:], rz_ps[:])
    rzbc_sb = persist.tile([P, 16], F32, tag="rzbc")
    nc.gpsimd.partition_broadcast(rzbc_sb[:], rzrow_sb[:1, :])
    wei_sb = persist.tile([P, 16], F32, tag="wei")
    nc.vector.tensor_mul(wei_sb[:], e_sb[:], rzbc_sb[:])
    nc.scalar.dma_start(io["wei_o"][:, :], wei_sb[:])

    # --- final: y_partial = ((P/Z) @ Wv_slice + bv_slice) @ Wp_slice (+bp) ---
    wx_sb = wx4_sb  # normalize in place: wx = P / Z (per-partition 1/Z scale)
    nc.vector.tensor_scalar_mul(wx_sb[:], wx4_sb[:], recip_sb[:, 0:1])
    wxT_sb = persist.tile([P, KK * B], F32, tag="wxT")
    for j in range(KK):
        tp_ps = psum.tile([P, B], F32, tag="ps")
        nc.tensor.transpose(tp_ps[:], wx_sb[:, j * P:(j + 1) * P], ident_sb[:])
        nc.vector.tensor_copy(wxT_sb[:, j * B:(j + 1) * B], tp_ps[:])

    tT_list = []
    if os.environ.get("K_TSTYLE", "narrow") == "wide":
        # t[b, c1] = sum_c wx[b, c] Wv[c, c1] as one [4, 256] accumulation
        t_ps = psum.tile([B, CS], F32, tag="ps")
        for k in range(KK):
            nc.tensor.matmul(
                t_ps[:],
                wxT_sb[:, k * B:(k + 1) * B],
                wv_sb[:, k * CS:(k + 1) * CS],
                start=(k == 0),
                stop=(k == KK - 1),
            )
        t_sb = persist.tile([B, CS], F32, tag="t")
        nc.scalar.copy(t_sb[:], t_ps[:])
        for m in range(2):
            tt_ps = psum.tile([P, B], F32, tag="ps", name=f"tt_ps_{m}")
            nc.tensor.transpose(tt_ps[:], t_sb[:, m * P:(m + 1) * P], ident_sb[:])
            tT_sb = persist.tile([P, B], F32, tag=f"tT{m}", name=f"tT_sb_{m}")
            nc.scalar.activation(
                tT_sb[:], tt_ps[:], AF.Identity, bias=bv_sb[:, m:m + 1], scale=1.0
            )
            tT_list.append(tT_sb)
    else:
        for m in range(2):
            t2_ps = psum.tile([P, B], F32, tag="ps", name=f"t2_ps_{m}")
            for k in range(KK):
                nc.tensor.matmul(
                    t2_ps[:],
                    wv_sb[:, k * CS + m * P: k * CS + m * P + P],
                    wxT_sb[:, k * B:(k + 1) * B],
                    start=(k == 0),
                    stop=(k == KK - 1),
                )
            tT_sb = persist.tile([P, B], F32, tag=f"tT{m}", name=f"tT_sb_{m}")
            nc.scalar.activation(
                tT_sb[:], t2_ps[:], AF.Identity, bias=bv_sb[:, m:m + 1], scale=1.0
            )
            tT_list.append(tT_sb)

    for n in range(4):
        y_ps = psum.tile([B, 512], F32, tag="ps", name=f"y_ps_{n}")
        for k in range(2):
            nc.tensor.matmul(
                y_ps[:],
                tT_list[k][:],
                wp_sb[:, k * C + n * 512: k * C + (n + 1) * 512],
                start=(k == 0),
                stop=(k == 1),
            )
        y_sb = psmall.tile([B, 512], F32, tag="yp", name=f"y_sb_{n}")
        nc.vector.tensor_add(y_sb[:], y_ps[:], bp_sb[:, n * 512:(n + 1) * 512])
        nc.scalar.dma_start(io["y_o"][:, n * 512:(n + 1) * 512], y_sb[:])


def build(n_devices=NC, with_cc=True):
    nc = bacc.Bacc(
        "TRN2",
        target_bir_lowering=False,
        debug=False,
        enable_asserts=True,
        num_devices=n_devices,
    )
    io = {}
    for name, shape in [
        ("xs", [B * TS, C]),
        ("xlastT", [C, B]),
        ("wq", [C, CS]),
        ("bqT", [CS, 1]),
        ("wkT", [CS, C]),
        ("wv", [C, CS]),
        ("bvT", [CS, 1]),
        ("wp", [CS, C]),
        ("bp_bc", [B, C]),
        ("sel", [16, B]),
        ("selT", [B, 16]),
    ]:
        io[name] = nc.dram_tensor(name, shape, F32, kind="ExternalInput").ap()
    for name, shape in [("y_o", [B, C]), ("wei_o", [P, 16])]:
        io[name] = nc.dram_tensor(name, shape, F32, kind="ExternalOutput").ap()

    with tile.TileContext(nc) as tc:
        with ExitStack() as ctx:
            _emit(nc, tc, ctx, io, with_cc, n_devices)
    nc.compile()
    return nc


_NC_CACHE = {}


def _get_nc():
    if "nc" not in _NC_CACHE:
        _NC_CACHE["nc"] = build()
    return _NC_CACHE["nc"]


def make_in_maps(x, Wk, bk, Wq, bq, Wv, bv, Wp, bp):
    x = np.ascontiguousarray(np.asarray(x, np.float32))
    xlastT = np.ascontiguousarray(x[:, -1, :].T)  # [C, B]
    sel = np.zeros((16, B), np.float32)
    for j in range(16):
        sel[j, j // 4] = 1.0
    selT = np.ascontiguousarray(sel.T)
    in_maps = []
    for i in range(NC):
        sl = slice(i * CS, (i + 1) * CS)
        in_maps.append({
            "xs": np.ascontiguousarray(
                x[:, i * TS:(i + 1) * TS, :].reshape(B * TS, C)
            ),
            "xlastT": xlastT,
            "wq": np.ascontiguousarray(np.asarray(Wq, np.float32)[:, sl]),
            "bqT": np.ascontiguousarray(np.asarray(bq, np.float32)[sl, None]),
            "wkT": np.ascontiguousarray(np.asarray(Wk, np.float32)[:, sl].T),
            "wv": np.ascontiguousarray(np.asarray(Wv, np.float32)[:, sl]),
            "bvT": np.ascontiguousarray(np.asarray(bv, np.float32)[sl, None]),
            "wp": np.ascontiguousarray(np.asarray(Wp, np.float32)[sl, :]),
            "bp_bc": (
                np.tile(np.asarray(bp, np.float32), (B, 1))
                if i == 0
                else np.zeros((B, C), np.float32)
            ),
            "sel": sel,
            "selT": selT,
        })
    return in_maps


def assemble_outputs(results):
    y = np.zeros((B, C), np.float64)
    wei = np.empty((B, T), np.float32)
    for i in range(NC):
        y += results[i]["y_o"].astype(np.float64)
        w = results[i]["wei_o"]  # [128, 16], col = b*4 + t4
        wei[:, i * TS:(i + 1) * TS] = (
            w.reshape(P, B, 4).transpose(1, 2, 0).reshape(B, TS)
        )
    out_last = y.astype(np.float32).reshape(B, 1, C)
    return out_last, wei.reshape(B, 1, T)


def kernel(**inputs):
    nc = _get_nc()
    in_maps = make_in_maps(**inputs)
    res = run_bass_kernel_spmd(nc, in_maps, core_ids=list(range(NC)))
    return assemble_outputs(res.results)


# revision 24
# speedup vs baseline: 3.6948x; 3.2514x over previous
"""Trainium2 Bass kernel for CausalSelfAttentionLast (last-query attention).

Reference math (per batch b):
    k = x @ Wk + bk                      [T, C]
    v = x @ Wv + bv                      [T, C]
    q = x[-1] @ Wq + bq                  [C]
    wei = softmax((q . k[t]) / sqrt(C))  [T]
    out = (wei @ v) @ Wp + bp            [C]

Algebraic restructuring (exact math, avoids the O(T*C^2) K/V projections):
    q . k[t] = x[t] . (Wk @ q) + q . bk
      -> u := Wk @ q  [C];  the q.bk term is constant in t and cancels in
         softmax, so bk never enters the computation at all.
    wei @ v = ((wei @ x) @ Wv) + (sum_t wei) * bv = (wei @ x) @ Wv + bv
    out = ((wei @ x) @ Wv + bv) @ Wp + bp

This reduces ~275 GFLOP of projections to ~0.3 GFLOP: a single streaming
pass over x computing logits[t] = scale * x[t].u and P[c] = sum_t e[t] x[t,c]
(unnormalized), plus tiny [4,C] x [C,C] matvec batches.

Sharding over 8 cores:
  - x is split along T (512 positions per core); each core computes its
    logits chunk and partial P/sum-of-exp.
  - the 4 CxC weights are split along the output (or contraction) dim in
    256-column slices per core; two 32KB AllReduces combine u and [P|Z].
  - final (wx @ Wv) @ Wp is sharded over the 256-wide inner dim; host sums
    the 8 partial outputs.
"""

import os
from contextlib import ExitStack

import numpy as np

import concourse.bass as bass
import concourse.tile as tile
from concourse import bacc, mybir
from concourse.bass_utils import run_bass_kernel_spmd
from concourse.masks import make_identity

B, T, C = 4, 4096, 2048
NC = 8
TS = T // NC        # 512 sequence positions per core
CS = C // NC        # 256-wide weight slice per core
KK = C // 128       # 16 contraction chunks of 128
P = 128
SCALE = float(C) ** -0.5
F32 = mybir.dt.float32
AF = mybir.ActivationFunctionType
ALU = mybir.AluOpType

# tuning knobs (env-overridable for cost-model sweeps; defaults are shipped)
XPOOL_BUFS = int(os.environ.get("K_XPOOL_BUFS", "12"))
SCRATCH_BUFS = int(os.environ.get("K_SCRATCH_BUFS", "1"))
ABL = set(os.environ.get("K_ABL", "").split(",")) - {""}
# float32r (single-pass fp32 matmul, 4x faster, reduced precision) for the
# P-accumulation matmuls; default off pending accuracy verification
USE_F32R = os.environ.get("K_F32R", "0") == "1"
F32R = mybir.dt.float32r


def _emit(nc, tc, ctx, io, with_cc, nd):
    persist = ctx.enter_context(tc.tile_pool(name="persist", bufs=1))
    psum = ctx.enter_context(tc.tile_pool(name="psum", bufs=8, space="PSUM"))
    dram = ctx.enter_context(tc.tile_pool(name="dram", bufs=1, space="DRAM"))
    xpool = ctx.enter_context(tc.tile_pool(name="xp", bufs=XPOOL_BUFS))
    scratch = ctx.enter_context(tc.tile_pool(name="scr", bufs=SCRATCH_BUFS))
    psmall = ctx.enter_context(tc.tile_pool(name="psb", bufs=2))

    def cc_allreduce(cin, cout):
        if with_cc:
            nc.gpsimd.collective_compute(
                "AllReduce",
                ALU.add,
                replica_groups=[list(range(nd))],
                ins=[cin.opt()],
                outs=[cout.opt()],
            )
        else:
            nc.gpsimd.dma_start(cout[:], cin[:])

    # --- small constants (SWDGE ring, out of the way of bulk loads) ---
    xlast_sb = persist.tile([P, KK * B], F32, tag="xlast")  # [c_chunk -> (k b)]
    nc.gpsimd.dma_start(
        xlast_sb[:].rearrange("p (k b) -> p k b", k=KK),
        io["xlastT"].rearrange("(k p) b -> p k b", p=P),
    )
    bq_sb = persist.tile([P, 2], F32, tag="bq")
    nc.gpsimd.dma_start(
        bq_sb[:].rearrange("p (m o) -> p m o", m=2),
        io["bqT"].rearrange("(m p) o -> p m o", p=P),
    )
    bv_sb = persist.tile([P, 2], F32, tag="bv")
    nc.gpsimd.dma_start(
        bv_sb[:].rearrange("p (m o) -> p m o", m=2),
        io["bvT"].rearrange("(m p) o -> p m o", p=P),
    )
    sel_sb = persist.tile([16, B], F32, tag="sel")
    nc.gpsimd.dma_start(sel_sb[:], io["sel"][:, :])
    selT_sb = persist.tile([B, 16], F32, tag="selT")
    nc.gpsimd.dma_start(selT_sb[:], io["selT"][:, :])
    bp_sb = persist.tile([B, C], F32, tag="bp")
    nc.gpsimd.dma_start(bp_sb[:], io["bp_bc"][:, :])
    ones_sb = persist.tile([P, 1], F32, tag="ones")
    nc.gpsimd.memset(ones_sb[:], 1.0)
    ident_sb = persist.tile([B, B], F32, tag="ident")
    make_identity(nc, ident_sb[:])
    e_sb = persist.tile([P, 16], F32, tag="e")  # exp(logits), col = b*4+t4

    # --- phase A: q = Wq^T x_last (transposed), u = Wk @ q, AllReduce u ---
    qT_list = []
    u_sb = persist.tile([B, C], F32, tag="u")
    with tc.tile_pool(name="wA", bufs=1) as wA:
        wq_sb = wA.tile([P, KK * CS], F32, tag="wq")
        for k in range(KK):
            nc.sync.dma_start(wq_sb[:, k * CS:(k + 1) * CS], io["wq"][k * P:(k + 1) * P, :])
        wkT_sb = wA.tile([P, 2 * C], F32, tag="wkT")
        for k in range(2):
            nc.sync.dma_start(wkT_sb[:, k * C:(k + 1) * C], io["wkT"][k * P:(k + 1) * P, :])

        if os.environ.get("K_QSTYLE", "narrow") == "wide":
            # q[b, c'] = sum_c x_last[b, c] Wq[c, c'] as one [4, 256] accum
            q_ps = psum.tile([B, CS], F32, tag="ps")
            for k in range(KK):
                nc.tensor.matmul(
                    q_ps[:],
                    xlast_sb[:, k * B:(k + 1) * B],
                    wq_sb[:, k * CS:(k + 1) * CS],
                    start=(k == 0),
                    stop=(k == KK - 1),
                )
            q_sb = persist.tile([B, CS], F32, tag="q")
            nc.scalar.copy(q_sb[:], q_ps[:])
            for m in range(2):
                qt_ps = psum.tile([P, B], F32, tag="ps", name=f"qt_ps_{m}")
                nc.tensor.transpose(qt_ps[:], q_sb[:, m * P:(m + 1) * P], ident_sb[:])
                qT_sb = persist.tile([P, B], F32, tag=f"qT{m}", name=f"qT_sb_{m}")
                # bias bq folds in after the transpose (per-partition there)
                nc.scalar.activation(
                    qT_sb[:], qt_ps[:], AF.Identity, bias=bq_sb[:, m:m + 1], scale=1.0
                )
                qT_list.append(qT_sb)
        else:
            for m in range(2):
                q_ps = psum.tile([P, B], F32, tag="ps", name=f"q_ps_{m}")
                for k in range(KK):
                    nc.tensor.matmul(
                        q_ps[:],
                        wq_sb[:, k * CS + m * P: k * CS + m * P + P],
                        xlast_sb[:, k * B:(k + 1) * B],
                        start=(k == 0),
                        stop=(k == KK - 1),
                    )
                qT_sb = persist.tile([P, B], F32, tag=f"qT{m}", name=f"qT_sb_{m}")
                nc.scalar.activation(
                    qT_sb[:], q_ps[:], AF.Identity, bias=bq_sb[:, m:m + 1], scale=1.0
                )
                qT_list.append(qT_sb)

        for n in range(4):
            u_ps = psum.tile([B, 512], F32, tag="ps")
            for k in range(2):
                nc.tensor.matmul(
                    u_ps[:],
                    qT_list[k][:],
                    wkT_sb[:, k * C + n * 512: k * C + (n + 1) * 512],
                    start=(k == 0),
                    stop=(k == 1),
                )
            nc.scalar.copy(u_sb[:, n * 512:(n + 1) * 512], u_ps[:])

    cc1_in = dram.tile([B, C], F32, tag="cc1i")
    cc1_out = dram.tile([B, C], F32, tag="cc1o")
    nc.gpsimd.dma_start(cc1_in[:], u_sb[:])
    cc_allreduce(cc1_in, cc1_out)

    # --- phase B: stream x; logits -> exp -> P accumulation ---
    cc2_in = dram.tile([1, B * C + B], F32, tag="cc2i")
    cc2_out = dram.tile([1, B * C + B], F32, tag="cc2o")
    poolB = None
    wv_sb = wp_sb = None
    for b in range(B):
        # bring back this b's row of the AllReduced u at partition 0, then
        # broadcast it across all 128 partitions for the DVE dot products
        urow_sb = psmall.tile([1, C], F32, tag="urow", name=f"urow_{b}")
        nc.gpsimd.dma_start(urow_sb[:], cc1_out[b:b + 1, :])
        u_bc = psmall.tile([P, C], F32, tag="ubc", name=f"ubc_{b}")
        nc.gpsimd.partition_broadcast(u_bc[:], urow_sb[:1, :])
        lg_sb = psmall.tile([P, 4], F32, tag="lg", name=f"lg_{b}")
        p_ps = [
            psum.tile([1, 512], F32, tag="ps", name=f"p_ps_{b}_{n}") for n in range(4)
        ]
        for t4 in range(4):
            tt = b * 4 + t4
            x_sb = xpool.tile([P, C], F32R if USE_F32R else F32, tag="x")
            nc.sync.dma_start(
                x_sb[:],
                io["xs"][tt * P:(tt + 1) * P, :].bitcast(F32R)
                if USE_F32R
                else io["xs"][tt * P:(tt + 1) * P, :],
            )
            x_f32 = x_sb[:].bitcast(F32) if USE_F32R else x_sb[:]
            if "nostt" not in ABL:
                tmp = scratch.tile([P, C], F32, tag="tmp")
                # fused (x * scale) * u with free-dim sum -> scaled logits
                # column (tensor_tensor_reduce is unsupported on this runtime)
                nc.vector.scalar_tensor_tensor(
                    out=tmp[:],
                    in0=x_f32,
                    scalar=SCALE,
                    in1=u_bc[:],
                    op0=ALU.mult,
                    op1=ALU.mult,
                    accum_out=lg_sb[:, t4:t4 + 1],
                )
            nc.scalar.activation(e_sb[:, tt:tt + 1], lg_sb[:, t4:t4 + 1], AF.Exp)
            if USE_F32R:
                er_col = psmall.tile([P, 1], F32R, tag="er", name=f"er_{tt}")
                nc.vector.tensor_copy(er_col[:], e_sb[:, tt:tt + 1])
                lhs_col = er_col[:]
            else:
                lhs_col = e_sb[:, tt:tt + 1]
            if "nopmm" not in ABL:
                for n in range(4):
                    nc.tensor.matmul(
                        p_ps[n][:],
                        lhs_col,
                        x_sb[:, n * 512:(n + 1) * 512],
                        start=(t4 == 0),
                        stop=(t4 == 3),
                    )
        pb_sb = psmall.tile([1, C], F32, tag="pb", bufs=1, name=f"pb_{b}")
        for n in range(4):
            nc.scalar.copy(pb_sb[:, n * 512:(n + 1) * 512], p_ps[n][:1, :])
        nc.scalar.dma_start(cc2_in[:, b * C:(b + 1) * C], pb_sb[:])

        if b == 0:
            # late bulk weight loads overlap the x stream on the same ring
            poolB = ctx.enter_context(tc.tile_pool(name="wB", bufs=1))
            wv_sb = poolB.tile([P, KK * CS], F32, tag="wv")
            for k in range(KK):
                nc.sync.dma_start(
                    wv_sb[:, k * CS:(k + 1) * CS], io["wv"][k * P:(k + 1) * P, :]
                )
            wp_sb = poolB.tile([P, 2 * C], F32, tag="wp")
            for k in range(2):
                nc.sync.dma_start(
                    wp_sb[:, k * C:(k + 1) * C], io["wp"][k * P:(k + 1) * P, :]
                )

    # --- sum of exp per b: column sums then fold groups of 4 ---
    cs_ps = psum.tile([16, 1], F32, tag="ps")
    nc.tensor.matmul(cs_ps[:], e_sb[:], ones_sb[:], start=True, stop=True)
    cs_sb = psmall.tile([16, 1], F32, tag="cs")
    nc.scalar.copy(cs_sb[:], cs_ps[:])
    srow_ps = psum.tile([1, B], F32, tag="ps")
    nc.tensor.matmul(srow_ps[:], cs_sb[:], sel_sb[:], start=True, stop=True)
    srow_sb = psmall.tile([1, B], F32, tag="srow")
    nc.scalar.copy(srow_sb[:], srow_ps[:])
    nc.scalar.dma_start(cc2_in[:, B * C:], srow_sb[:])

    cc_allreduce(cc2_in, cc2_out)
    wx4_sb = persist.tile([B, C], F32, tag="wx4")
    nc.scalar.dma_start(
        wx4_sb[:], cc2_out[:, :B * C].rearrange("o (b c) -> (o b) c", b=B)
    )
    z4_sb = persist.tile([B, 1], F32, tag="z4")
    nc.scalar.dma_start(
        z4_sb[:], cc2_out[:, B * C:].rearrange("o (b c) -> (o b) c", b=B)
    )

    # --- wei output: e / Z ---
    recip_sb = persist.tile([B, 1], F32, tag="rz")
    nc.vector.reciprocal(recip_sb[:], z4_sb[:])
    rz_ps = psum.tile([1, 16], F32, tag="ps")
    nc.tensor.matmul(rz_ps[:], recip_sb[:], selT_sb[:], start=True, stop=True)
    rzrow_sb = psmall.tile([1, 16], F32, tag="rzrow")
    nc.scalar.copy(rzrow_sb[:], rz_ps[:])
    rzbc_sb = persist.tile([P, 16], F32, tag="rzbc")
    nc.gpsimd.partition_broadcast(rzbc_sb[:], rzrow_sb[:1, :])
    wei_sb = persist.tile([P, 16], F32, tag="wei")
    nc.vector.tensor_mul(wei_sb[:], e_sb[:], rzbc_sb[:])
    nc.scalar.dma_start(io["wei_o"][:, :], wei_sb[:])

    # --- final: y_partial = ((P/Z) @ Wv_slice + bv_slice) @ Wp_slice (+bp) ---
    wx_sb = wx4_sb  # normalize in place: wx = P / Z (per-partition 1/Z scale)
    nc.vector.tensor_scalar_mul(wx_sb[:], wx4_sb[:], recip_sb[:, 0:1])
    wxT_sb = persist.tile([P, KK * B], F32, tag="wxT")
    for j in range(KK):
        tp_ps = psum.tile([P, B], F32, tag="ps")
        nc.tensor.transpose(tp_ps[:], wx_sb[:, j * P:(j + 1) * P], ident_sb[:])
        nc.vector.tensor_copy(wxT_sb[:, j * B:(j + 1) * B], tp_ps[:])

    tT_list = []
    if os.environ.get("K_TSTYLE", "narrow") == "wide":
        # t[b, c1] = sum_c wx[b, c] Wv[c, c1] as one [4, 256] accumulation
        t_ps = psum.tile([B, CS], F32, tag="ps")
        for k in range(KK):
            nc.tensor.matmul(
                t_ps[:],
                wxT_sb[:, k * B:(k + 1) * B],
                wv_sb[:, k * CS:(k + 1) * CS],
                start=(k == 0),
                stop=(k == KK - 1),
            )
        t_sb = persist.tile([B, CS], F32, tag="t")
        nc.scalar.copy(t_sb[:], t_ps[:])
        for m in range(2):
            tt_ps = psum.tile([P, B], F32, tag="ps", name=f"tt_ps_{m}")
            nc.tensor.transpose(tt_ps[:], t_sb[:, m * P:(m + 1) * P], ident_sb[:])
            tT_sb = persist.tile([P, B], F32, tag=f"tT{m}", name=f"tT_sb_{m}")
            nc.scalar.activation(
                tT_sb[:], tt_ps[:], AF.Identity, bias=bv_sb[:, m:m + 1], scale=1.0
            )
            tT_list.append(tT_sb)
    else:
        for m in range(2):
            t2_ps = psum.tile([P, B], F32, tag="ps", name=f"t2_ps_{m}")
            for k in range(KK):
                nc.tensor.matmul(
                    t2_ps[:],
                    wv_sb[:, k * CS + m * P: k * CS + m * P + P],
                    wxT_sb[:, k * B:(k + 1) * B],
                    start=(k == 0),
                    stop=(k == KK - 1),
                )
            tT_sb = persist.tile([P, B], F32, tag=f"tT{m}", name=f"tT_sb_{m}")
            nc.scalar.activation(
                tT_sb[:], t2_ps[:], AF.Identity, bias=bv_sb[:, m:m + 1], scale=1.0
            )
            tT_list.append(tT_sb)

    for n in range(4):
        y_ps = psum.tile([B, 512], F32, tag="ps", name=f"y_ps_{n}")
        for k in range(2):
            nc.tensor.matmul(
                y_ps[:],
                tT_list[k][:],
                wp_sb[:, k * C + n * 512: k * C + (n + 1) * 512],
                start=(k == 0),
                stop=(k == 1),
            )
        y_sb = psmall.tile([B, 512], F32, tag="yp", name=f"y_sb_{n}")
        nc.vector.tensor_add(y_sb[:], y_ps[:], bp_sb[:, n * 512:(n + 1) * 512])
        nc.scalar.dma_start(io["y_o"][:, n * 512:(n + 1) * 512], y_sb[:])


def build(n_devices=NC, with_cc=True):
    nc = bacc.Bacc(
        "TRN2",
        target_bir_lowering=False,
        debug=False,
        enable_asserts=True,
        num_devices=n_devices,
    )
    io = {}
    for name, shape in [
        ("xs", [B * TS, C]),
        ("xlastT", [C, B]),
        ("wq", [C, CS]),
        ("bqT", [CS, 1]),
        ("wkT", [CS, C]),
        ("wv", [C, CS]),
        ("bvT", [CS, 1]),
        ("wp", [CS, C]),
        ("bp_bc", [B, C]),
        ("sel", [16, B]),
        ("selT", [B, 16]),
    ]:
        io[name] = nc.dram_tensor(name, shape, F32, kind="ExternalInput").ap()
    for name, shape in [("y_o", [B, C]), ("wei_o", [P, 16])]:
        io[name] = nc.dram_tensor(name, shape, F32, kind="ExternalOutput").ap()

    with tile.TileContext(nc) as tc:
        with ExitStack() as ctx:
            _emit(nc, tc, ctx, io, with_cc, n_devices)
    nc.compile()
    return nc


_NC_CACHE = {}


def _get_nc():
    if "nc" not in _NC_CACHE:
        _NC_CACHE["nc"] = build()
    return _NC_CACHE["nc"]


def make_in_maps(x, Wk, bk, Wq, bq, Wv, bv, Wp, bp):
    x = np.ascontiguousarray(np.asarray(x, np.float32))
    xlastT = np.ascontiguousarray(x[:, -1, :].T)  # [C, B]
    sel = np.zeros((16, B), np.float32)
    for j in range(16):
        sel[j, j // 4] = 1.0
    selT = np.ascontiguousarray(sel.T)
    in_maps = []
    for i in range(NC):
        sl = slice(i * CS, (i + 1) * CS)
        in_maps.append({
            "xs": np.ascontiguousarray(
                x[:, i * TS:(i + 1) * TS, :].reshape(B * TS, C)
            ),
            "xlastT": xlastT,
            "wq": np.ascontiguousarray(np.asarray(Wq, np.float32)[:, sl]),
            "bqT": np.ascontiguousarray(np.asarray(bq, np.float32)[sl, None]),
            "wkT": np.ascontiguousarray(np.asarray(Wk, np.float32)[:, sl].T),
            "wv": np.ascontiguousarray(np.asarray(Wv, np.float32)[:, sl]),
            "bvT": np.ascontiguousarray(np.asarray(bv, np.float32)[sl, None]),
            "wp": np.ascontiguousarray(np.asarray(Wp, np.float32)[sl, :]),
            "bp_bc": (
                np.tile(np.asarray(bp, np.float32), (B, 1))
                if i == 0
                else np.zeros((B, C), np.float32)
            ),
            "sel": sel,
            "selT": selT,
        })
    return in_maps


def assemble_outputs(results):
    y = np.zeros((B, C), np.float64)
    wei = np.empty((B, T), np.float32)
    for i in range(NC):
        y += results[i]["y_o"].astype(np.float64)
        w = results[i]["wei_o"]  # [128, 16], col = b*4 + t4
        wei[:, i * TS:(i + 1) * TS] = (
            w.reshape(P, B, 4).transpose(1, 2, 0).reshape(B, TS)
        )
    out_last = y.astype(np.float32).reshape(B, 1, C)
    return out_last, wei.reshape(B, 1, T)


def kernel(**inputs):
    nc = _get_nc()
    in_maps = make_in_maps(**inputs)
    res = run_bass_kernel_spmd(nc, in_maps, core_ids=list(range(NC)))
    return assemble_outputs(res.results)
